# revision 1
# baseline (speedup 1.0000x reference)
"""Trainium2 Bass kernel for GCN-biased sparse attention (nn_Attention_37589553775245).

Reference computation (per batch b of 8, one NeuronCore each):
    qkv = x @ w_qkv; q,k,v per head (H=8, DH=64)
    attn = softmax(q k^T / sqrt(DH)) + A_hat        (A_hat = D^-1/2 (ceil(adj)+I) D^-1/2)
    out = (attn @ v) @ w_out + b_out

Sharding: pure batch-parallel across the 8 cores (B=8). A_hat is computed on
host (cheap) and replicated; weights replicated. No collectives.

Device-side layout strategy (all matmuls in float32r — tf32-class precision,
1 cycle/row at free-dim 512):
  - x is pre-transposed on host to xT [DIM, N] so the first matmul contraction
    (over DIM) sits on the partition axis.
  - q,k are produced transposed (qT,kT [DH, N]); v in natural [N, F] layout.
  - scores are computed transposed: sT[j,i] = sum_d k[j,d] qT[d,i], so the
    softmax denominator (sum over j) rides the attn@v matmul via an augmented
    V with a ones column: [v_h | 1] gives out rows 0..63 = (exp sT)^T v and
    row 64 = sum_j exp sT[j,i] (the denominator). Softmax max-subtraction is
    skipped: logits*scale ~ N(0,1), exp is safe in fp32.
  - adjacent heads (2h, 2h+1) sit at partition bases 0/64 of one qkT tile;
    q/k tiles are produced in head-pair order so early heads unblock first.
  - attention units are woven INTO phase 1 (head-0/1 q,k tiles + v first,
    remaining q/k tiles emitted between attention units) so the ACT engine's
    exp stream — the longest single-engine load, ~78 us — starts ~20 us
    earlier and the PE always has independent projection work while waiting;
    normalized attention outputs are written once into a dedicated yE tensor.
  - A_hat^T's 4MB SBUF residency is deferred: its tile allocates from the
    space freed by xT/w_qkv after phase 1, which is what lets the attention
    pools coexist with the phase-1 buffers under the 192KB/partition budget.
  - post-softmax bias contribution A_hat @ V is its own matmul (shared over
    heads), computed transposed via lhsT=V tiles, rhs=A_hat^T (host-shipped),
    woven between the late attention units (fully decoupled from the
    attention tails via yE) so the PE works while ACT drains the last exps.
  - Y^T = yE + (A_hat V)^T is merged per f-tile, then out = Y @ w_out + b_out
    with lhsT = Y^T tiles.
"""

import os
import sys

import numpy as np

for _p in ("/opt/trn_rl_repo", "/root/.axon_site/_ro/trn_rl_repo"):
    if _p not in sys.path and os.path.isdir(_p):
        sys.path.insert(0, _p)

import concourse.bass as bass  # noqa: E402
import concourse.mybir as mybir  # noqa: E402
import concourse.tile as tile  # noqa: E402
from concourse import bacc  # noqa: E402
from concourse.bass_utils import run_bass_kernel_spmd  # noqa: E402

B, N, DIM, H, DH = 8, 1024, 512, 8, 64
F = H * DH          # 512, inner dim
NT = N // 128       # 8 n-tiles (also j-tiles)
DT = DIM // 128     # 4 dim-tiles
FT = F // 128       # 4 f-tiles
NC2 = N // 512      # 2 i-chunks of 512
SCALE = DH ** -0.5

F32 = mybir.dt.float32
F32R = mybir.dt.float32r

_PROGRAM = None
_last_in_maps = None


def _build_program(reps=1, copies_dve=True, exp_batch=2, interleave=False,
                   pair_heads=False, skip_exp=False, skip_attn=False,
                   mm_bufs=2, s_bufs=None, o_bufs=None, exps_bufs=6,
                   early_attn=True):
    nc = bacc.Bacc("TRN2", target_bir_lowering=False, debug=False, num_devices=8)

    xT_d = nc.dram_tensor("xT", [DIM, N], F32R, kind="ExternalInput")
    wqkv_d = nc.dram_tensor("wqkv", [DIM, 3 * F], F32R, kind="ExternalInput")
    ahatT_d = nc.dram_tensor("ahatT", [N, N], F32R, kind="ExternalInput")
    wout_d = nc.dram_tensor("wout", [F, DIM], F32R, kind="ExternalInput")
    bout_d = nc.dram_tensor("bout", [1, DIM], F32, kind="ExternalInput")
    out_d = nc.dram_tensor("out", [N, DIM], F32, kind="ExternalOutput")

    def copy_out(dst, src):
        if copies_dve:
            nc.vector.tensor_copy(out=dst, in_=src)
        else:
            nc.scalar.copy(out=dst, in_=src)

    # PSUM budget is 8 banks total; a [128, 512] fp32 tile is one bank.
    if pair_heads:
        exp_batch = 1       # score tiles stay 1 bank; 2 in flight per jb
        s_bufs = 3 if s_bufs is None else s_bufs
        o_bufs = 3 if o_bufs is None else o_bufs
    else:
        s_bufs = 2 if s_bufs is None else s_bufs
        o_bufs = 2 if o_bufs is None else o_bufs
    with tile.TileContext(nc) as tc:
        with (
            tc.tile_pool(name="big", bufs=1) as big,
            tc.tile_pool(name="ps_mm", bufs=mm_bufs, space="PSUM") as ps_mm,
            tc.tile_pool(name="ps_s", bufs=s_bufs, space="PSUM") as ps_s,
            tc.tile_pool(name="ps_o", bufs=o_bufs, space="PSUM") as ps_o,
        ):
          for _rep in range(reps):
            # ---- persistent SBUF tensors -------------------------------
            # (ahatT is allocated later, reusing phase-1's freed SBUF)
            wout = big.tile([128, FT, DIM], F32R)
            qkT = big.tile([128, 2 * FT, N], F32R)       # [f, n] f=q(0:512),k(512:1024)
            v_sb = big.tile([128, NT, F], F32R)          # v[n, f]
            vaug = big.tile([128, NT, H, DH + 1], F32R)  # [n, h, v|1]
            yT = big.tile([128, FT, N], F32R)            # Y^T[f, i] (A_hat V part)
            yE = big.tile([128, FT, N], F32R)            # Y^T (exp-attention part)
            bout_bc = big.tile([128, DIM], F32)

            nc.vector.memset(vaug.bitcast(F32), 1.0)  # ones col survives v copies

            # attention-unit pools open BEFORE phase 1 so attention can be
            # emitted interleaved with the projections (fits SBUF because
            # ahatT's 32KB/partition is deferred past the ph1 release)
            exps = tc.alloc_tile_pool(name="exps", bufs=exps_bufs)
            small = tc.alloc_tile_pool(name="small", bufs=2)
            outs = tc.alloc_tile_pool(name="outs", bufs=3)
            dscr = tc.alloc_tile_pool(name="dscr", bufs=4, space="DRAM")

            # ---- phase 1: qT/kT (transposed) and v (natural) -----------
            ph1 = tc.alloc_tile_pool(name="ph1", bufs=1)
            if True:
                xT = ph1.tile([128, DT, N], F32R)        # xT[dim, n]
                wqkv = ph1.tile([128, DT, 3 * F], F32R)
                # phase-1-gating loads go first on the sync HWDGE ring (FIFO);
                # the big A_hat^T load rides the idle SWDGE ring in parallel.
                nc.sync.dma_start(
                    out=xT,
                    in_=xT_d[:, :].rearrange("(t p) n -> p t n", p=128),
                )
                for fc in range(6):   # 256-col chunks so early groups unblock
                    nc.sync.dma_start(
                        out=wqkv[:, :, fc * 256:(fc + 1) * 256],
                        in_=wqkv_d[:, fc * 256:(fc + 1) * 256].rearrange(
                            "(t p) f -> p t f", p=128),
                    )
                nc.sync.dma_start(
                    out=wout,
                    in_=wout_d[:, :].rearrange("(t p) n -> p t n", p=128),
                )
                nc.sync.dma_start(out=bout_bc, in_=bout_d[0:1, :].to_broadcast((128, DIM)))
            def emit_qk(ft):
                for c in range(NC2):
                    ps = ps_mm.tile([128, 512], F32, tag="mm")
                    for dt_i in range(DT):
                        nc.tensor.matmul(
                            ps,
                            wqkv[:, dt_i, ft * 128:(ft + 1) * 128],
                            xT[:, dt_i, c * 512:(c + 1) * 512],
                            start=(dt_i == 0),
                            stop=(dt_i == DT - 1),
                        )
                    copy_out(qkT[:, ft, c * 512:(c + 1) * 512], ps)

            def emit_v():
                for nt in range(NT):
                    ps = ps_mm.tile([128, 512], F32, tag="mm")
                    for dt_i in range(DT):
                        nc.tensor.matmul(
                            ps,
                            xT[:, dt_i, nt * 128:(nt + 1) * 128],
                            wqkv[:, dt_i, 2 * F:3 * F],
                            start=(dt_i == 0),
                            stop=(dt_i == DT - 1),
                        )
                    copy_out(v_sb[:, nt, :], ps)
                    nc.vector.tensor_copy(
                        out=vaug[:, nt, :, 0:DH],
                        in_=ps.rearrange("p (h d) -> p h d", h=H),
                    )

            def ahat_unit(ft, c):
                # (A_hat @ V)^T [f-tile ft, i-chunk c] -> yT
                ps = ps_mm.tile([128, 512], F32, tag="mm")
                for jt in range(NT):
                    nc.tensor.matmul(
                        ps,
                        v_sb[:, jt, ft * 128:(ft + 1) * 128],
                        ahatT[:, jt, c * 512:(c + 1) * 512],
                        start=(jt == 0),
                        stop=(jt == NT - 1),
                    )
                copy_out(yT[:, ft, c * 512:(c + 1) * 512], ps)

            def attn_tail(h, c, ps_out):
                # normalize expv^T by the ridden denominator; write-once into
                # yE (merged with the A_hat V part in yT before projection)
                hb = (h % 2) * 64
                recip = small.tile([65, 512], F32, tag="recip")
                nc.vector.reciprocal(out=recip[64:65, :], in_=ps_out[64:65, :])
                # partition-broadcast via DRAM bounce (SBUF sources can't
                # broadcast across partitions; DRAM sources can)
                scr = dscr.tile([1, 512], F32, tag="scr")
                nc.sync.dma_start(out=scr, in_=recip[64:65, :])
                bcast = small.tile([64, 512], F32, tag="bcast")
                nc.sync.dma_start(out=bcast, in_=scr.to_broadcast((64, 512)))
                ysl = yE[hb:hb + 64, h // 2, c * 512:(c + 1) * 512]
                if hb == 0:
                    nc.vector.tensor_mul(ysl, ps_out[0:64, :], bcast)
                else:
                    # DVE lanes can't shift partitions; write at base 0 and
                    # SWDGE-copy into the base-64 yE slice.
                    prod = small.tile([64, 512], F32R, tag="prod")
                    nc.vector.tensor_mul(prod, ps_out[0:64, :], bcast)
                    nc.gpsimd.dma_start(out=ysl, in_=prod)

            def attn_unit(h, c):
                # one head, one 512-wide i-chunk
                hb = (h % 2) * 64
                ht = h // 2
                ps_out = ps_o.tile([65, 512], F32, tag="po")
                for jb in range(NT // exp_batch):
                    ps_sc = ps_s.tile([128, exp_batch, 512], F32, tag="ps")
                    for e in range(exp_batch):
                        jt = jb * exp_batch + e
                        # scoresT[j, i] = sum_d kT[d, j] qT[d, i]
                        nc.tensor.matmul(
                            ps_sc[:, e, :],
                            qkT[hb:hb + 64, FT + ht, jt * 128:(jt + 1) * 128],
                            qkT[hb:hb + 64, ht, c * 512:(c + 1) * 512],
                        )
                    et = exps.tile([128, exp_batch, 512], F32R, tag="exp")
                    if skip_exp:
                        nc.vector.tensor_copy(out=et, in_=ps_sc)
                    else:
                        nc.scalar.activation(
                            out=et, in_=ps_sc,
                            func=mybir.ActivationFunctionType.Exp,
                            scale=float(SCALE),
                        )
                    for e in range(exp_batch):
                        jt = jb * exp_batch + e
                        # [expv^T ; denom] accumulation
                        nc.tensor.matmul(
                            ps_out,
                            vaug[:, jt, h, :],
                            et[:, e, :],
                            start=(jt == 0),
                            stop=(jt == NT - 1),
                        )
                attn_tail(h, c, ps_out)

            def attn_unit_pair(hp, c):
                # heads 2hp (rows 0:64) and 2hp+1 (rows 64:128) share qkT
                # tiles; their score matmuls hit disjoint PE row groups and
                # run concurrently.
                ht = hp
                po = [ps_o.tile([65, 512], F32, tag="po", name=f"po{u}")
                      for u in range(2)]
                for jb in range(NT // exp_batch):
                    ps_sc = [ps_s.tile([128, exp_batch, 512], F32, tag="ps",
                                       name=f"ps_sc{u}")
                             for u in range(2)]
                    for e in range(exp_batch):
                        jt = jb * exp_batch + e
                        for u, hb in enumerate((0, 64)):
                            nc.tensor.matmul(
                                ps_sc[u][:, e, :],
                                qkT[hb:hb + 64, FT + ht, jt * 128:(jt + 1) * 128],
                                qkT[hb:hb + 64, ht, c * 512:(c + 1) * 512],
                            )
                    ets = []
                    for u in range(2):
                        et = exps.tile([128, exp_batch, 512], F32R, tag="exp",
                                       name=f"et{u}")
                        if skip_exp:
                            nc.vector.tensor_copy(out=et, in_=ps_sc[u])
                        else:
                            nc.scalar.activation(
                                out=et, in_=ps_sc[u],
                                func=mybir.ActivationFunctionType.Exp,
                                scale=float(SCALE),
                            )
                        ets.append(et)
                    for e in range(exp_batch):
                        jt = jb * exp_batch + e
                        for u in range(2):
                            nc.tensor.matmul(
                                po[u],
                                vaug[:, jt, 2 * hp + u, :],
                                ets[u][:, e, :],
                                start=(jt == 0),
                                stop=(jt == NT - 1),
                            )
                for u in range(2):
                    attn_tail(2 * hp + u, c, po[u])

            # ---- emission: phase 1 woven with attention ----------------
            # head-0/1 q,k tiles + all of v first, then attention units with
            # the remaining q/k tiles woven between them (PE gets independent
            # work while ACT chews the exp stream, which starts ~20us sooner).
            emit_qk(0)
            emit_qk(4)
            emit_v()
            rest_qk = [1, 5, 2, 6, 3, 7]
            ahat_left = [(ft, c) for ft in range(FT) for c in range(NC2)]
            ahatT = None

            def emit_ahat():
                ft, c = ahat_left.pop(0)
                ahat_unit(ft, c)
                if c == NC2 - 1 and not skip_attn:
                    # merge Y^T parts per f-tile; DVE overlaps later A_hat MMs
                    nc.vector.tensor_add(yT[:, ft, :], yT[:, ft, :],
                                         yE[:, ft, :])

            if skip_attn:
                for ft in rest_qk:
                    emit_qk(ft)
                ph1.release()
                ahatp = tc.alloc_tile_pool(name="ahatp", bufs=1)
                ahatT = ahatp.tile([128, NT, N], F32R)
                nc.sync.dma_start(
                    out=ahatT,
                    in_=ahatT_d[:, :].rearrange("(t p) n -> p t n", p=128),
                )
                while ahat_left:
                    emit_ahat()
            else:
                qi = 0
                for u, (h, c) in enumerate([(h, c) for h in range(H)
                                            for c in range(NC2)]):
                    attn_unit(h, c)
                    if qi < len(rest_qk):
                        emit_qk(rest_qk[qi])
                        qi += 1
                        if qi == len(rest_qk):
                            # phase-1 buffers die here; A_hat^T lands in the
                            # freed SBUF while attention continues
                            ph1.release()
                            ahatp = tc.alloc_tile_pool(name="ahatp", bufs=1)
                            ahatT = ahatp.tile([128, NT, N], F32R)
                            nc.sync.dma_start(
                                out=ahatT,
                                in_=ahatT_d[:, :].rearrange(
                                    "(t p) n -> p t n", p=128),
                            )
                    elif u >= 10:
                        # weave A_hat units between late attention units so
                        # the PE works while ACT drains the last exp batches
                        emit_ahat()
                while ahat_left:
                    emit_ahat()

            # ---- phase 4: out = Y @ w_out + b_out ----------------------
            for nt in range(NT):
                ps = ps_mm.tile([128, 512], F32, tag="mm")
                for ft in range(FT):
                    nc.tensor.matmul(
                        ps,
                        yT[:, ft, nt * 128:(nt + 1) * 128],
                        wout[:, ft, :],
                        start=(ft == 0),
                        stop=(ft == FT - 1),
                    )
                ot = outs.tile([128, DIM], F32, tag="ot")
                nc.vector.tensor_add(ot, ps, bout_bc)
                nc.sync.dma_start(out=out_d[nt * 128:(nt + 1) * 128, :], in_=ot)

            ahatp.release()
            dscr.release()
            outs.release()
            small.release()
            exps.release()

    nc.compile()
    return nc


def _get_program():
    global _PROGRAM
    if _PROGRAM is None:
        _PROGRAM = _build_program()
    return _PROGRAM


def kernel(x, adj, w_qkv, w_out, b_out):
    x = np.asarray(x, dtype=np.float32)
    adj = np.asarray(adj, dtype=np.float32)
    w_qkv = np.ascontiguousarray(np.asarray(w_qkv, dtype=np.float32))
    w_out = np.ascontiguousarray(np.asarray(w_out, dtype=np.float32))
    b_out = np.asarray(b_out, dtype=np.float32).reshape(1, DIM)

    # host-side: normalized adjacency bias, replicated (cheap: one 1024^2 pass)
    A = np.ceil(adj) + np.eye(N, dtype=np.float32)
    dinv = A.sum(axis=1) ** -0.5
    A_hat = (A * dinv[:, None]) * dinv[None, :]
    ahatT = np.ascontiguousarray(A_hat.T)

    nc = _get_program()
    in_maps = []
    for b in range(B):
        in_maps.append({
            "xT": np.ascontiguousarray(x[b].T),
            "wqkv": w_qkv,
            "ahatT": ahatT,
            "wout": w_out,
            "bout": b_out,
        })
    global _last_in_maps
    _last_in_maps = in_maps
    res = run_bass_kernel_spmd(nc, in_maps, list(range(B)))
    out = np.stack([res.results[b]["out"] for b in range(B)], axis=0)
    return out.astype(np.float32)


if __name__ == "__main__":
    rng = np.random.default_rng(0)
    x = rng.standard_normal((B, N, DIM), dtype=np.float32)
    adj = (rng.random((N, N), dtype=np.float32) < 0.05).astype(np.float32) * 0.5
    w_qkv = rng.standard_normal((DIM, 3 * F), dtype=np.float32) * DIM ** -0.5
    w_out = rng.standard_normal((F, DIM), dtype=np.float32) * F ** -0.5
    b_out = np.zeros(DIM, dtype=np.float32)
    out = kernel(x=x, adj=adj, w_qkv=w_qkv, w_out=w_out, b_out=b_out)
    print("out", out.shape, out.dtype, np.abs(out).max())



# revision 16
# speedup vs baseline: 24.9516x; 24.9516x over previous
"""Trainium2 Bass kernel for GCN-biased sparse attention (nn_Attention_37589553775245).

Reference computation (per batch b of 8, one NeuronCore each):
    qkv = x @ w_qkv; q,k,v per head (H=8, DH=64)
    attn = softmax(q k^T / sqrt(DH)) + A_hat        (A_hat = D^-1/2 (ceil(adj)+I) D^-1/2)
    out = (attn @ v) @ w_out + b_out

Sharding: pure batch-parallel across the 8 cores (B=8). A_hat is computed on
host (cheap) and replicated; weights replicated. No collectives.

Layout strategy (all matmul operands bf16, PSUM accumulation fp32; at 2e-2
tolerance bf16 is comfortably safe and it halves DMA/SBUF vs fp32r):
  - x pre-transposed on host to xT [DIM, N]; first matmul contraction (DIM)
    on the partition axis. q,k produced transposed (qT,kT [DH,N], head pairs
    at partition bases 0/64); v natural [N,F] into a per-head augmented
    [v_h | 1] tile (the ones column rides the attn@v matmul to produce the
    softmax denominator on the output partition axis).
  - scores transposed: sT[j,i] = sum_d k[j,d] qT[d,i] ([128j, 512i] tiles,
    1 PSUM bank); ACT exp (scale folded) -> bf16 exp tiles. Softmax
    max-subtraction skipped: logits*scale ~ N(0,1), exp safe in fp32.
  - attn@v FLIPPED to natural orientation: lhsT = exp-tile [128j, 128i],
    rhs = vaug_h [128j, 65] -> PSUM [128 i, 64 d | denom]. Uses all 128 PE
    output partitions (the old transposed form used 65/128), and the
    denominator lands on the partition axis so normalization is a cheap DVE
    reciprocal + tensor_scalar_mul (no partition-broadcast DRAM bounce).
  - (A_hat V)^T computed transposed (lhsT = v-natural strided view of vaug,
    rhs = A_hat^T tiles) late, as PE filler; it is merged during the
    Y-transpose copy-out (DVE add), so it never gates the attention pipeline.
  - Y (normalized attention, natural [i,f]) is PE-transposed per [128,128]
    tile (bf16 transpose = 128 cycles) into Y^T for the out projection;
    out = Y^T-tiles.T @ w_out + b_out.
  - unit order c-outer/h-inner: i-chunk 0 finishes all heads halfway through,
    so its transposes + out projections + stores weave into chunk 1's
    attention units; only chunk 1's tail runs after the last exp.
  - qkT PSUM->SBUF copies ride the Pool (gpsimd) engine; DVE keeps v/avT
    copies, normalization, transpose-merges, bias adds.
"""

import os
import sys

import numpy as np

for _p in ("/opt/trn_rl_repo", "/root/.axon_site/_ro/trn_rl_repo"):
    if _p not in sys.path and os.path.isdir(_p):
        sys.path.insert(0, _p)

import concourse.bass as bass  # noqa: E402
import concourse.mybir as mybir  # noqa: E402
import concourse.tile as tile  # noqa: E402
from concourse import bacc  # noqa: E402
from concourse.bass_utils import run_bass_kernel_spmd  # noqa: E402
from concourse.masks import make_identity  # noqa: E402

B, N, DIM, H, DH = 8, 1024, 512, 8, 64
F = H * DH          # 512, inner dim
NT = N // 128       # 8 n-tiles (also j-tiles)
DT = DIM // 128     # 4 dim-tiles
FT = F // 128       # 4 f-tiles
NC2 = N // 512      # 2 i-chunks of 512
IS = 512 // 128     # 4 i-subtiles per chunk
SCALE = DH ** -0.5

F32 = mybir.dt.float32
BF16 = mybir.dt.bfloat16

_PROGRAM = None
_last_in_maps = None


def _build_program(reps=1):
    nc = bacc.Bacc("TRN2", target_bir_lowering=False, debug=False, num_devices=8)

    xT_d = nc.dram_tensor("xT", [DIM, N], BF16, kind="ExternalInput")
    wqkv_d = nc.dram_tensor("wqkv", [DIM, 3 * F], BF16, kind="ExternalInput")
    ahatT_d = nc.dram_tensor("ahatT", [N, N], BF16, kind="ExternalInput")
    wout_d = nc.dram_tensor("wout", [F, DIM], BF16, kind="ExternalInput")
    bout_d = nc.dram_tensor("bout", [1, DIM], F32, kind="ExternalInput")
    out_d = nc.dram_tensor("out", [N, DIM], F32, kind="ExternalOutput")

    with tile.TileContext(nc) as tc:
        with (
            tc.tile_pool(name="big", bufs=1) as big,
            tc.tile_pool(name="ps_mm", bufs=2, space="PSUM") as ps_mm,
            tc.tile_pool(name="ps_s", bufs=2, space="PSUM") as ps_s,
            tc.tile_pool(name="ps_av", bufs=2, space="PSUM") as ps_av,
            tc.tile_pool(name="exps", bufs=6) as exps,
            tc.tile_pool(name="small", bufs=4) as small,
            tc.tile_pool(name="outs", bufs=3) as outs,
        ):
          for _rep in range(reps):
            # ---- persistent SBUF tensors -------------------------------
            xT = big.tile([128, DT, N], BF16)
            wqkv = big.tile([128, DT, 3 * F], BF16)
            wout = big.tile([128, FT, DIM], BF16)
            qkT = big.tile([128, 2 * FT, N], BF16)       # q(ft 0:4), k(ft 4:8)
            vaug = big.tile([128, NT, H, DH + 1], BF16)  # [j, jt, h, v|1]
            v_sb = big.tile([128, NT, F], BF16)          # v natural [j, f]
            ahatT = big.tile([128, NT, N], BF16)         # A_hat^T [j, i]
            yN = big.tile([128, NT, F], BF16)            # Y natural [i, f]
            avT = big.tile([128, FT, N], BF16)           # (A_hat V)^T [f, i]
            yT = big.tile([128, FT, N], BF16)            # Y^T [f, i]
            ident = big.tile([128, 128], BF16)
            bout_bc = big.tile([128, DIM], F32)
            warm = big.tile([1, 8], F32)

            make_identity(nc, ident)
            nc.vector.memset(vaug[:, :, :, DH:DH + 1], 1.0)  # denominator ones
            # warm the ACT exp table during the load phase
            nc.vector.memset(warm, 0.0)
            nc.scalar.activation(out=warm, in_=warm,
                                 func=mybir.ActivationFunctionType.Exp)

            # ---- input DMAs (ordered by first use; DMA engines serialize
            # heavily, so the critical-path tensors must go first) ---------
            def load_wqkv(fc):
                nc.sync.dma_start(
                    out=wqkv[:, :, fc * 256:(fc + 1) * 256],
                    in_=wqkv_d[:, fc * 256:(fc + 1) * 256].rearrange(
                        "(t p) f -> p t f", p=128),
                )

            def load_xT(c):
                nc.sync.dma_start(
                    out=xT[:, :, c * 512:(c + 1) * 512],
                    in_=xT_d[:, c * 512:(c + 1) * 512].rearrange(
                        "(t p) n -> p t n", p=128))

            load_wqkv(0)          # q heads 0-3
            load_xT(0)
            load_wqkv(2)          # k heads 0-3
            load_wqkv(4)          # v cols 0-255
            load_wqkv(5)          # v cols 256-511
            load_xT(1)
            load_wqkv(1)          # q heads 4-7
            load_wqkv(3)          # k heads 4-7
            nc.sync.dma_start(
                out=wout, in_=wout_d[:, :].rearrange("(t p) n -> p t n", p=128))
            nc.sync.dma_start(out=bout_bc,
                              in_=bout_d[0:1, :].to_broadcast((128, DIM)))
            # big A_hat^T load last: first needed ~halfway through; on the
            # sync ring its transfer queues behind all the hot loads
            nc.sync.dma_start(
                out=ahatT,
                in_=ahatT_d[:, :].rearrange("(t p) n -> p t n", p=128),
            )

            # ---- projection / filler units -----------------------------
            def emit_qk1(ft, c):
                # qkT f-tile ft (transposed), one 512-wide n-chunk
                ps = ps_mm.tile([128, 512], F32, tag="mm", name="ps_qk")
                for dt_i in range(DT):
                    nc.tensor.matmul(
                        ps,
                        wqkv[:, dt_i, ft * 128:(ft + 1) * 128],
                        xT[:, dt_i, c * 512:(c + 1) * 512],
                        start=(dt_i == 0),
                        stop=(dt_i == DT - 1),
                    )
                nc.vector.tensor_copy(
                    out=qkT[:, ft, c * 512:(c + 1) * 512], in_=ps)

            def emit_qk(ft):
                for c in range(NC2):
                    emit_qk1(ft, c)

            def emit_v1(nt):
                # v natural [n, f] into vaug (per-head columns + ones)
                ps = ps_mm.tile([128, 512], F32, tag="mm", name="ps_v")
                for dt_i in range(DT):
                    nc.tensor.matmul(
                        ps,
                        xT[:, dt_i, nt * 128:(nt + 1) * 128],
                        wqkv[:, dt_i, 2 * F:3 * F],
                        start=(dt_i == 0),
                        stop=(dt_i == DT - 1),
                    )
                nc.vector.tensor_copy(
                    out=vaug[:, nt, :, 0:DH],
                    in_=ps.rearrange("p (h d) -> p h d", h=H),
                )
                nc.vector.tensor_copy(out=v_sb[:, nt, :], in_=ps)

            ahat_ps = {}

            def ahat_unit(ft, c, half):
                # (A_hat V)^T [f-tile ft, i-chunk c] -> avT (half a j-sweep)
                if half == 0:
                    ahat_ps[(ft, c)] = ps_mm.tile([128, 512], F32, tag="mm",
                                                  name="ps_ah")
                ps = ahat_ps[(ft, c)]
                for jt in range(half * 4, half * 4 + 4):
                    nc.tensor.matmul(
                        ps,
                        v_sb[:, jt, ft * 128:(ft + 1) * 128],
                        ahatT[:, jt, c * 512:(c + 1) * 512],
                        start=(jt == 0),
                        stop=(jt == NT - 1),
                    )
                if half == 1:
                    nc.vector.tensor_copy(
                        out=avT[:, ft, c * 512:(c + 1) * 512],
                        in_=ahat_ps.pop((ft, c)))

            def trans_unit(ft, it):
                # yT[f, i-tile it] = yN[it, f-tile ft]^T + avT (merged add)
                ps = ps_mm.tile([128, 512], F32, tag="mm", name="ps_tr")
                trv = ps.bitcast(BF16)[:, 0:128]
                nc.tensor.transpose(
                    trv, yN[:, it, ft * 128:(ft + 1) * 128], ident)
                nc.vector.tensor_add(
                    yT[:, ft, it * 128:(it + 1) * 128],
                    trv,
                    avT[:, ft, it * 128:(it + 1) * 128],
                )

            def out_unit(nt):
                # out[i-tile nt] = Y^T-tiles.T @ w_out + b_out
                ps = ps_mm.tile([128, 512], F32, tag="mm")
                for ft in range(FT):
                    nc.tensor.matmul(
                        ps,
                        yT[:, ft, nt * 128:(nt + 1) * 128],
                        wout[:, ft, :],
                        start=(ft == 0),
                        stop=(ft == FT - 1),
                    )
                ot = outs.tile([128, DIM], F32, tag="ot")
                nc.vector.tensor_add(ot, ps, bout_bc)
                nc.sync.dma_start(out=out_d[nt * 128:(nt + 1) * 128, :], in_=ot)

            # ---- attention: flat 2-deep software pipeline --------------
            # stage k = (c, h, jb): scores+exp issued at k, attn@v at k-2,
            # so the PE never sits on an exp dependency without queued work
            # and the ACT exp stream is continuously fed.
            JB = NT // 2
            stages = [(c, h, jb)
                      for c in range(NC2) for h in range(H) for jb in range(JB)]
            sc_of = {}
            et_of = {}
            pav_of = {}

            def emit_scores(k):
                c, h, jb = stages[k]
                hb = (h % 2) * 64
                ht = h // 2
                if jb == 0:
                    pav_of[(c, h)] = ps_av.tile([128, IS, DH + 1], F32,
                                                tag="av", name="pav")
                ps_sc = ps_s.tile([128, 2, 512], F32, tag="ps", name="ps_sc")
                for e in range(2):
                    jt = jb * 2 + e
                    # scoresT[j, i] = sum_d kT[d, j] qT[d, i]
                    nc.tensor.matmul(
                        ps_sc[:, e, :],
                        qkT[hb:hb + 64, FT + ht, jt * 128:(jt + 1) * 128],
                        qkT[hb:hb + 64, ht, c * 512:(c + 1) * 512],
                    )
                et = exps.tile([128, 2, 512], BF16, tag="exp", name="et")
                nc.scalar.activation(
                    out=et, in_=ps_sc,
                    func=mybir.ActivationFunctionType.Exp,
                    scale=float(SCALE),
                )
                sc_of[k] = ps_sc
                et_of[k] = et

            def emit_av(k):
                c, h, jb = stages[k]
                et = et_of.pop(k)
                pav = pav_of[(c, h)]
                for e in range(2):
                    jt = jb * 2 + e
                    for isub in range(IS):
                        # natural orientation: [128 i, v|1]. start=True zeroes
                        # the whole 2KB psum bank (zero region), so only the
                        # unit's first matmul starts; later isubs' first
                        # writes overwrite pending-zero bytes (has_written).
                        nc.tensor.matmul(
                            pav[:, isub, :],
                            et[:, e, isub * 128:(isub + 1) * 128],
                            vaug[:, jt, h, :],
                            start=(jt == 0 and isub == 0),
                            stop=(jt == NT - 1 and isub == IS - 1),
                        )
                if jb == JB - 1:
                    # normalize: denominators ride the free axis, one per isub
                    pav = pav_of.pop((c, h))
                    recip = small.tile([128, IS], F32, tag="recip")
                    nc.vector.reciprocal(out=recip, in_=pav[:, :, DH:DH + 1])
                    for isub in range(IS):
                        it = c * IS + isub
                        nc.vector.tensor_scalar_mul(
                            out=yN[:, it, h * DH:(h + 1) * DH],
                            in0=pav[:, isub, 0:DH],
                            scalar1=recip[:, isub:isub + 1],
                        )

            # ---- fill steps: small PE work items woven between stages --
            # Deadlines (PE is in-order; a score emitted at iter k-2 must
            # have its qkT inputs earlier in program order):
            #   v j-tiles 2k,2k+1 before av(stage jb=k)     -> idx <= k
            #   head h's q/k chunks before scores(4h)       -> idx <= 4h-3
            #   (k chunk c covers j-range jb in {2c, 2c+1} only)
            fill = {}

            def tr_group(ft, c):
                for it in range(c * IS, c * IS + IS):
                    trans_unit(ft, it)

            def emit_v2(nt):
                emit_v1(nt)
                emit_v1(nt + 1)

            for i, nt in enumerate((2, 4, 6)):             # dl: av(nt//2)
                fill[i] = lambda nt=nt: emit_v2(nt)
            for i, a in enumerate(((1, 0), (5, 0), (5, 1), (2, 0), (6, 0),
                                   (6, 1), (3, 0), (7, 0), (7, 1))):
                fill[3 + i] = lambda a=a: emit_qk1(*a)     # dl 5,5,7,13,...
            for ft in range(FT):   # (A_hat V)^T chunk 0: dl 31 (tr c0 @32)
                fill[21 + 2 * ft] = lambda ft=ft: ahat_unit(ft, 0, 0)
                fill[22 + 2 * ft] = lambda ft=ft: ahat_unit(ft, 0, 1)
            for i, ft in enumerate(range(FT)):             # tr c0: dl-bound
                fill[32 + i] = lambda ft=ft: tr_group(ft, 0)
            fill[36] = lambda: emit_qk1(1, 1)              # dl 37
            fill[37] = lambda: ahat_unit(0, 1, 0)          # dl 39 (tr @40)
            fill[38] = lambda: ahat_unit(0, 1, 1)
            fill[39] = lambda: out_unit(0)
            fill[40] = lambda: tr_group(0, 1)
            fill[41] = lambda: out_unit(1)
            fill[42] = lambda: out_unit(2)
            fill[43] = lambda: out_unit(3)
            fill[44] = lambda: emit_qk1(2, 1)              # dl 45
            fill[45] = lambda: ahat_unit(1, 1, 0)          # dl 47 (tr @48)
            fill[46] = lambda: ahat_unit(1, 1, 1)
            fill[48] = lambda: tr_group(1, 1)
            fill[52] = lambda: emit_qk1(3, 1)              # dl 53
            fill[53] = lambda: ahat_unit(2, 1, 0)          # dl 55 (tr @56)
            fill[54] = lambda: ahat_unit(2, 1, 1)
            fill[56] = lambda: tr_group(2, 1)
            fill[60] = lambda: ahat_unit(3, 1, 0)          # dl: epilogue
            fill[61] = lambda: ahat_unit(3, 1, 1)

            # ---- emission ----------------------------------------------
            # pre-phase: only the c0 chunks gate the first scores; the c1
            # k-chunk must still precede scores(2) in PE program order
            emit_qk1(0, 0)      # q heads 0,1 chunk c0
            emit_qk1(4, 0)      # k heads 0,1 j-tiles 0-3
            NS = len(stages)
            emit_scores(0)
            emit_scores(1)
            emit_v1(0)
            emit_v1(1)
            emit_qk1(4, 1)      # k heads 0,1 j-tiles 4-7 (scores(2,3))
            emit_qk1(0, 1)      # q heads 0,1 chunk c1 (stage 32)
            for k in range(NS):
                if k + 2 < NS:
                    emit_scores(k + 2)
                if k in fill:
                    fill[k]()
                emit_av(k)
            # chunk-1 epilogue
            tr_group(3, 1)
            for nt in range(IS, NT):
                out_unit(nt)

    nc.compile()
    return nc


def _get_program():
    global _PROGRAM
    if _PROGRAM is None:
        _PROGRAM = _build_program()
    return _PROGRAM


def kernel(x, adj, w_qkv, w_out, b_out):
    bf16 = mybir.dt.np(BF16)
    x = np.asarray(x, dtype=np.float32)
    adj = np.asarray(adj, dtype=np.float32)
    w_qkv = np.ascontiguousarray(np.asarray(w_qkv, dtype=np.float32)).astype(bf16)
    w_out = np.ascontiguousarray(np.asarray(w_out, dtype=np.float32)).astype(bf16)
    b_out = np.asarray(b_out, dtype=np.float32).reshape(1, DIM)

    # host-side: normalized adjacency bias, replicated (cheap: one 1024^2 pass)
    A = np.ceil(adj) + np.eye(N, dtype=np.float32)
    dinv = A.sum(axis=1) ** -0.5
    A_hat = (A * dinv[:, None]) * dinv[None, :]
    ahatT = np.ascontiguousarray(A_hat.T).astype(bf16)

    nc = _get_program()
    in_maps = []
    for b in range(B):
        in_maps.append({
            "xT": np.ascontiguousarray(x[b].T).astype(bf16),
            "wqkv": w_qkv,
            "ahatT": ahatT,
            "wout": w_out,
            "bout": b_out,
        })
    global _last_in_maps
    _last_in_maps = in_maps
    res = run_bass_kernel_spmd(nc, in_maps, list(range(B)))
    out = np.stack([res.results[b]["out"] for b in range(B)], axis=0)
    return out.astype(np.float32)


if __name__ == "__main__":
    rng = np.random.default_rng(0)
    x = rng.standard_normal((B, N, DIM), dtype=np.float32)
    adj = (rng.random((N, N), dtype=np.float32) < 0.05).astype(np.float32) * 0.5
    w_qkv = rng.standard_normal((DIM, 3 * F), dtype=np.float32) * DIM ** -0.5
    w_out = rng.standard_normal((F, DIM), dtype=np.float32) * F ** -0.5
    b_out = np.zeros(DIM, dtype=np.float32)
    out = kernel(x=x, adj=adj, w_qkv=w_qkv, w_out=w_out, b_out=b_out)
    print("out", out.shape, out.dtype, np.abs(out).max())
